# revision 8
# baseline (speedup 1.0000x reference)
"""Multi-head attention (B=4, N=2048, C=1024, H=16, D=64) on 8 TRN2 NeuronCores.

Sharding: data-parallel over batch (4) x tensor-parallel over heads (2 groups
of 8 heads).  Device d handles batch d//2 and head-group d%2.  Each device:
  qT/kT = Wq/Wk-slice @ x^T   (channels-on-partitions layout)
  V     = x @ Wv-slice^T      (keys-on-partitions layout, +ones column)
  per head: S[k,q] = kT^T-chunks x qT, E = exp(S/8),
            [O^T; denom] = [V|1]^T @ E  (accumulated over key chunks)
            O^T /= denom  (gpsimd partition-broadcast of reciprocal)
  Y^T_partial = proj_w-slice^T-chunks @ O^T (+bias on group-0 devices)
Host sums the two partials per batch and transposes back.

All matmuls run as float32r (FP22 multiply, FP32 accumulate).
"""

import os
import sys

for _p in ("/opt/trn_rl_repo", "/root/.axon_site/_ro/trn_rl_repo"):
    if os.path.isdir(_p) and _p not in sys.path:
        sys.path.insert(0, _p)

import numpy as np

B, N, C = 4, 2048, 1024
H_LOC = 8  # heads per device
D = 64
CH = 512  # qkv channels per device (H_LOC * D)
P = 128
SCALE = 0.125  # D ** -0.5
NKC = N // P  # 16 key chunks
NQC = N // 512  # 4 query chunks of 512
NCI = C // P  # 8 c_in chunks
NPAIR = 4  # head pairs per device

_CACHE = {}
LAST_EXEC_TIME_NS = None


def _build():
    import concourse.bacc as bacc
    import concourse.mybir as mybir
    import concourse.tile as tile

    F32 = mybir.dt.float32
    F32R = mybir.dt.float32r
    Exp = mybir.ActivationFunctionType.Exp

    nc = bacc.Bacc("TRN2", target_bir_lowering=False, debug=False)

    xT_d = nc.dram_tensor("xT", [C, N], F32R, kind="ExternalInput")
    wq_d = nc.dram_tensor("wq", [C, CH], F32R, kind="ExternalInput")
    wk_d = nc.dram_tensor("wk", [C, CH], F32R, kind="ExternalInput")
    wv_d = nc.dram_tensor("wv", [C, CH], F32R, kind="ExternalInput")
    pw_d = nc.dram_tensor("pw", [CH, C], F32R, kind="ExternalInput")
    bias_d = nc.dram_tensor("bias", [C], F32, kind="ExternalInput")
    yT_d = nc.dram_tensor("yT", [C, N], F32, kind="ExternalOutput")

    xT_re = xT_d[:].rearrange("(c p) n -> p c n", p=P)
    wq_re = wq_d[:].rearrange("(c p) m -> p c m", p=P)
    wk_re = wk_d[:].rearrange("(c p) m -> p c m", p=P)
    wv_re = wv_d[:].rearrange("(c p) m -> p c m", p=P)
    pw_re = pw_d[:].rearrange("(c p) m -> p c m", p=P)
    bias_re = bias_d[:].rearrange("(a p) -> p a", p=P)
    yT_re = yT_d[:].rearrange("(a p) n -> p a n", p=P)

    with tile.TileContext(nc) as tc:
        with (
            tc.tile_pool(name="ps_s", bufs=2, space="PSUM") as ps_s,
            tc.tile_pool(name="ps_o", bufs=1, space="PSUM") as ps_o,
            tc.tile_pool(name="ps_mm", bufs=2, space="PSUM") as ps_mm,
            tc.tile_pool(name="dramp", bufs=1, space="DRAM") as dramp,
        ):
            ot_dram = dramp.tile([CH, N], F32R)
            ot_out_re = ot_dram.rearrange("(c p) n -> p c n", p=P)

            with (
                tc.tile_pool(name="persist", bufs=1) as persist,
                tc.tile_pool(name="xp", bufs=1) as xp,
                tc.tile_pool(name="qkp", bufs=2) as qkp,
                tc.tile_pool(name="wqkp", bufs=1) as wqkp,
                tc.tile_pool(name="ep", bufs=3) as ep,
            ):
                xsb = xp.tile([P, NCI, N], F32R)
                for nb in range(NQC):
                    sl = slice(nb * 512, (nb + 1) * 512)
                    nc.sync.dma_start(out=xsb[:, :, sl], in_=xT_re[:, :, sl])

                # ---- V pass: V[k, ch] with interleaved ones columns ----
                v_sb = persist.tile([P, NKC, H_LOC * 65], F32R)
                ones_view = v_sb.rearrange("p k (h e) -> p k h e", e=65)[
                    :, :, :, 64:65
                ]
                ones_src = persist.tile([P, NKC, H_LOC], F32)
                nc.vector.memset(ones_src, 1.0)
                nc.vector.tensor_copy(
                    ones_view, ones_src.rearrange("p k (h o) -> p k h o", o=1)
                )
                oc = persist.tile([1, D], F32R)
                nc.vector.tensor_copy(
                    oc, ones_src.rearrange("p k h -> p (k h)")[0:1, 0:D]
                )
                with tc.tile_pool(name="wvp", bufs=1) as wvp:
                    wv_sb = wvp.tile([P, NCI, CH], F32R)
                    nc.sync.dma_start(out=wv_sb, in_=wv_re)
                    for kc in range(NKC):
                        v_ps = ps_mm.tile([P, CH], F32, tag="mm")
                        ksl = slice(kc * P, (kc + 1) * P)
                        for c in range(NCI):
                            nc.tensor.matmul(
                                v_ps,
                                lhsT=xsb[:, c, ksl],
                                rhs=wv_sb[:, c, :],
                                start=(c == 0),
                                stop=(c == NCI - 1),
                            )
                        nc.vector.tensor_copy(
                            v_sb.rearrange("p k (h e) -> p k h e", e=65)[
                                :, kc, :, 0:64
                            ],
                            v_ps.rearrange("p (h e) -> p h e", e=64),
                        )

                # ---- per head-pair: q/k projection then attention ----
                for t in range(NPAIR):
                    csl = slice(t * P, (t + 1) * P)
                    wq_sb = wqkp.tile([P, NCI, P], F32R, tag="wq")
                    wk_sb = wqkp.tile([P, NCI, P], F32R, tag="wk")
                    nc.sync.dma_start(out=wq_sb, in_=wq_re[:, :, csl])
                    nc.sync.dma_start(out=wk_sb, in_=wk_re[:, :, csl])
                    qT_t = qkp.tile([P, N], F32R, tag="qT")
                    kT_t = qkp.tile([P, N], F32R, tag="kT")
                    for nb in range(NQC):
                        nsl = slice(nb * 512, (nb + 1) * 512)
                        q_ps = ps_mm.tile([P, 512], F32, tag="mm")
                        for c in range(NCI):
                            nc.tensor.matmul(
                                q_ps,
                                lhsT=wq_sb[:, c, :],
                                rhs=xsb[:, c, nsl],
                                start=(c == 0),
                                stop=(c == NCI - 1),
                            )
                        nc.vector.tensor_copy(qT_t[:, nsl], q_ps)
                        k_ps = ps_mm.tile([P, 512], F32, tag="mm")
                        for c in range(NCI):
                            nc.tensor.matmul(
                                k_ps,
                                lhsT=wk_sb[:, c, :],
                                rhs=xsb[:, c, nsl],
                                start=(c == 0),
                                stop=(c == NCI - 1),
                            )
                        nc.vector.tensor_copy(kT_t[:, nsl], k_ps)

                    for qc in range(NQC):
                        qsl = slice(qc * 512, (qc + 1) * 512)
                        o0 = ps_o.tile([65, 512], F32, tag="o0")
                        o1 = ps_o.tile([65, 512], F32, tag="o1")
                        for kc in range(NKC):
                            ksl = slice(kc * P, (kc + 1) * P)
                            s = ps_s.tile([P, 1024], F32, tag="s")
                            nc.tensor.matmul(
                                s[:, 0:512],
                                lhsT=kT_t[0:D, ksl],
                                rhs=qT_t[0:D, qsl],
                                start=True,
                                stop=True,
                            )
                            nc.tensor.matmul(
                                s[:, 512:1024],
                                lhsT=kT_t[D:P, ksl],
                                rhs=qT_t[D:P, qsl],
                                start=True,
                                stop=True,
                            )
                            e = ep.tile([P, 1024], F32R, tag="e")
                            nc.scalar.activation(e, s, Exp, scale=SCALE)
                            for hh, o_ps in ((0, o0), (1, o1)):
                                h = 2 * t + hh
                                nc.tensor.matmul(
                                    o_ps,
                                    lhsT=v_sb[:, kc, 65 * h : 65 * h + 65],
                                    rhs=e[:, 512 * hh : 512 * hh + 512],
                                    start=(kc == 0),
                                    stop=(kc == NKC - 1),
                                )
                        for hh, o_ps in ((0, o0), (1, o1)):
                            o_sb = ep.tile([65, 512], F32, tag="osb", bufs=2)
                            nc.vector.tensor_copy(o_sb, o_ps)
                            rd = ep.tile([1, 512], F32R, tag="rd", bufs=2)
                            nc.vector.reciprocal(
                                o_sb[64:65, :], o_sb[64:65, :]
                            )
                            nc.vector.tensor_copy(rd, o_sb[64:65, :])
                            rb = ps_mm.tile([64, 512], F32, tag="mm")
                            nc.tensor.matmul(
                                rb, lhsT=oc, rhs=rd, start=True, stop=True
                            )
                            ostg = ep.tile([64, 512], F32R, tag="ostg", bufs=2)
                            nc.vector.tensor_mul(ostg, o_sb[0:64, :], rb)
                            nc.sync.dma_start(
                                out=ot_out_re[64 * hh : 64 * hh + 64, t, qsl],
                                in_=ostg,
                            )

            # ---- output projection: yT = pw^T-chunks @ OT (+bias) ----
            with tc.tile_pool(name="projp", bufs=1) as projp:
                pw_sb = projp.tile([P, NPAIR, C], F32R)
                nc.sync.dma_start(out=pw_sb, in_=pw_re)
                bias_sb = projp.tile([P, NCI], F32)
                nc.sync.dma_start(out=bias_sb, in_=bias_re)
                ot_in = projp.tile([P, NPAIR, N], F32R)
                nc.sync.dma_start(out=ot_in, in_=ot_out_re)
                with tc.tile_pool(name="yp", bufs=3) as yp:
                    for co in range(NCI):
                        cosl = slice(co * P, (co + 1) * P)
                        for ns in range(NQC):
                            nsl = slice(ns * 512, (ns + 1) * 512)
                            y_ps = ps_mm.tile([P, 512], F32, tag="mm")
                            for ci in range(NPAIR):
                                nc.tensor.matmul(
                                    y_ps,
                                    lhsT=pw_sb[:, ci, cosl],
                                    rhs=ot_in[:, ci, nsl],
                                    start=(ci == 0),
                                    stop=(ci == NPAIR - 1),
                                )
                            y_sb = yp.tile([P, 512], F32, tag="y")
                            nc.vector.tensor_scalar(
                                y_sb,
                                y_ps,
                                bias_sb[:, co : co + 1],
                                None,
                                op0=mybir.AluOpType.add,
                            )
                            nc.sync.dma_start(out=yT_re[:, co, nsl], in_=y_sb)

    nc.compile()
    return nc


def get_nc():
    if "nc" not in _CACHE:
        _CACHE["nc"] = _build()
    return _CACHE["nc"]


def make_in_maps(x, qkv_w, proj_w, proj_b):
    x = np.asarray(x, dtype=np.float32)
    qkv_w = np.asarray(qkv_w, dtype=np.float32)
    proj_w = np.asarray(proj_w, dtype=np.float32)
    proj_b = np.asarray(proj_b, dtype=np.float32)
    in_maps = []
    for d in range(8):
        b, g = d // 2, d % 2
        gs = slice(CH * g, CH * (g + 1))
        in_maps.append(
            {
                "xT": np.ascontiguousarray(x[b].T),
                "wq": np.ascontiguousarray(qkv_w[0 * C :][gs.start : gs.stop].T),
                "wk": np.ascontiguousarray(qkv_w[1 * C :][gs.start : gs.stop].T),
                "wv": np.ascontiguousarray(qkv_w[2 * C :][gs.start : gs.stop].T),
                "pw": np.ascontiguousarray(proj_w[:, gs].T),
                "bias": proj_b if g == 0 else np.zeros_like(proj_b),
            }
        )
    return in_maps


def kernel(x, qkv_w, proj_w, proj_b):
    global LAST_EXEC_TIME_NS
    from concourse import bass_utils

    nc = get_nc()
    in_maps = make_in_maps(x, qkv_w, proj_w, proj_b)
    res = bass_utils.run_bass_kernel_spmd(
        nc, in_maps, core_ids=list(range(8))
    )
    LAST_EXEC_TIME_NS = res.exec_time_ns
    out = np.empty((B, N, C), dtype=np.float32)
    for b in range(B):
        out[b] = (res.results[2 * b]["yT"] + res.results[2 * b + 1]["yT"]).T
    return out
